# revision 27
# baseline (speedup 1.0000x reference)
"""Block-diagonal linear (grouped GEMM) on 8 TRN2 NeuronCores.

out[b, g*512+n] = sum_k x[b, g*512+k] * blocks[g, k, n]

Sharding: group-parallel — core g computes block g's GEMM. The host hands
each core xT = x[:, g*512:(g+1)*512].T ([512, 8192], feature-major) and
receives outT ([512, 8192]); the transposes happen on the host so the
device needs no PE transposes and every DMA stream reads/writes long
contiguous runs per partition.

Per-core kernel: out.T = W.T @ x.T as 64 PSUM accumulation groups:
psum[n-tile 128, m 512] += W[k-tile, n-tile].T @ xT[k-tile, m-chunk],
with all matmul operands rounded to float32r (full PE rate at N=512,
~1.5e-4 max rel err vs fp32).
"""
import numpy as np

import concourse.bacc as bacc
import concourse.tile as tile
from concourse import mybir
from concourse.bass_utils import run_bass_kernel_spmd

TOKENS = 8192
G = 8
M = 512  # per-block in-features
N = 512  # per-block out-features
P = 128
KT = M // P  # 4 contraction tiles
NT = N // P  # 4 output feature tiles
SUB = 512    # tokens per PSUM group (moving-dim max for 4-byte dtypes)
F32 = mybir.dt.float32
F32R = mybir.dt.float32r

# token-chunk schedule: small head/tail for pipeline ramp, 2048 steady
CHUNKS = [512, 512, 1024, 2048, 2048, 1024, 512, 512]
assert sum(CHUNKS) == TOKENS
CMAX = max(CHUNKS)

_CACHE: dict = {}


def _body(tc, nc, xT, w, outT):
    with (
        tc.tile_pool(name="wp", bufs=1) as wp,
        tc.tile_pool(name="xin", bufs=10) as xin,
        tc.tile_pool(name="outp", bufs=2) as outp,
        tc.tile_pool(name="pso", bufs=8, space="PSUM") as pso,
    ):
        # weights [512, 512] -> [128, kt, 512] fp32, rounded once to f32r
        w_f = wp.tile([P, KT, N], F32, tag="wf")
        w_r = wp.tile([P, KT, N], F32R, tag="wr")
        w_v = w.rearrange("(j p) n -> j p n", p=P)

        m0 = 0
        for ci, c in enumerate(CHUNKS):
            # load + round the 4 k-tiles of this token chunk, striped across
            # the two HWDGE rings (sync=SP and scalar=ACT)
            xs = []
            for j in range(KT):
                x_t = xin.tile([P, CMAX], F32R, tag="x")
                eng = nc.sync if j % 2 == 0 else nc.scalar
                eng.dma_start(
                    x_t[:, :c], xT[j * P:(j + 1) * P, m0:m0 + c].bitcast(F32R)
                )
                # round to f32r in place (read+write same AP is per-element safe)
                nc.vector.tensor_copy(x_t[:, :c], x_t[:, :c])
                xs.append(x_t)
            if ci == 0:
                # W rides both rings right behind the first chunk
                for j in range(KT):
                    eng = nc.sync if j % 2 == 0 else nc.scalar
                    eng.dma_start(w_f[:, j, :], w_v[j])
                    nc.vector.tensor_copy(w_r[:, j, :], w_f[:, j, :])

            ots = [outp.tile([P, CMAX], F32, tag=f"o{nt}", name=f"ot{nt}") for nt in range(NT)]
            for s0 in range(0, c, SUB):
                sw = min(SUB, c - s0)
                for nt in range(NT):
                    ps_o = pso.tile([P, SUB], F32, tag="pso")
                    for j in range(KT):
                        nc.tensor.matmul(
                            ps_o[:, :sw],
                            w_r[:, j, nt * P:(nt + 1) * P],
                            xs[j][:, s0:s0 + sw],
                            start=(j == 0),
                            stop=(j == KT - 1),
                        )
                    nc.scalar.copy(ots[nt][:, s0:s0 + sw], ps_o[:, :sw])
            # flush the chunk: one DMA per n-tile on the SWDGE ring; the last
            # chunks ride the HWDGE rings (input traffic is done by then)
            for nt in range(NT):
                if ci >= len(CHUNKS) - 3:
                    eng = nc.sync if nt % 2 == 0 else nc.scalar
                else:
                    eng = nc.gpsimd
                eng.dma_start(outT[nt * P:(nt + 1) * P, m0:m0 + c], ots[nt][:, :c])
            m0 += c


def _build():
    nc = bacc.Bacc("TRN2", target_bir_lowering=False, debug=False, num_devices=G)
    xT = nc.dram_tensor("xT", [M, TOKENS], F32, kind="ExternalInput").ap()
    w = nc.dram_tensor("w", [M, N], F32, kind="ExternalInput").ap()
    outT = nc.dram_tensor("outT", [N, TOKENS], F32, kind="ExternalOutput").ap()
    with tile.TileContext(nc) as tc:
        _body(tc, nc, xT, w, outT)
    nc.compile()
    return nc


def _run(in_maps, **kwargs):
    if "nc" not in _CACHE:
        _CACHE["nc"] = _build()
    return run_bass_kernel_spmd(_CACHE["nc"], in_maps, list(range(G)), **kwargs)


def _in_maps(x, blocks):
    return [
        {
            "xT": np.ascontiguousarray(x[:, g * M:(g + 1) * M].T, dtype=np.float32),
            "w": np.ascontiguousarray(blocks[g], dtype=np.float32),
        }
        for g in range(G)
    ]


def kernel(x, blocks):
    x = np.asarray(x)
    blocks = np.asarray(blocks)
    res = _run(_in_maps(x, blocks))
    return np.concatenate(
        [res.results[g]["outT"].T for g in range(G)], axis=1
    ).astype(np.float32, copy=False)


# revision 28
# speedup vs baseline: 1.1405x; 1.1405x over previous
"""Block-diagonal linear (grouped GEMM) on 8 TRN2 NeuronCores.

out[b, g*512+n] = sum_k x[b, g*512+k] * blocks[g, k, n]

Sharding: group-parallel — core g computes block g's GEMM. The host hands
each core xT = x[:, g*512:(g+1)*512].T ([512, 8192], feature-major) and
receives outT ([512, 8192]); the transposes happen on the host so the
device needs no PE transposes and every DMA stream reads/writes long
contiguous runs per partition.

Per-core kernel: out.T = W.T @ x.T as 64 PSUM accumulation groups:
psum[n-tile 128, m 512] += W[k-tile, n-tile].T @ xT[k-tile, m-chunk],
with all matmul operands rounded to float32r (full PE rate at N=512,
~1.5e-4 max rel err vs fp32).
"""
import numpy as np

import concourse.bacc as bacc
import concourse.tile as tile
from concourse import mybir
from concourse.bass_utils import run_bass_kernel_spmd

TOKENS = 8192
G = 8
M = 512  # per-block in-features
N = 512  # per-block out-features
P = 128
KT = M // P  # 4 contraction tiles
NT = N // P  # 4 output feature tiles
SUB = 512    # tokens per PSUM group (moving-dim max for 4-byte dtypes)
F32 = mybir.dt.float32
F32R = mybir.dt.float32r

# token-chunk schedule: small head/tail for pipeline ramp, 2048 steady
CHUNKS = [512, 512, 1024, 2048, 2048, 1024, 512, 512]
assert sum(CHUNKS) == TOKENS
CMAX = max(CHUNKS)

_CACHE: dict = {}


def _body(tc, nc, xT, w, outT):
    with (
        tc.tile_pool(name="wp", bufs=1) as wp,
        tc.tile_pool(name="xin", bufs=10) as xin,
        tc.tile_pool(name="outp", bufs=2) as outp,
        tc.tile_pool(name="pso", bufs=8, space="PSUM") as pso,
    ):
        # weights [512, 512] -> [128, kt, 512] fp32, rounded once to f32r
        w_f = wp.tile([P, KT, N], F32, tag="wf")
        w_r = wp.tile([P, KT, N], F32R, tag="wr")
        w_v = w.rearrange("(j p) n -> j p n", p=P)

        m0 = 0
        for ci, c in enumerate(CHUNKS):
            # load + round the 4 k-tiles of this token chunk, striped across
            # the two HWDGE rings (sync=SP and scalar=ACT)
            xs = []
            for j in range(KT):
                x_t = xin.tile([P, CMAX], F32R, tag="x")
                eng = nc.sync if j % 2 == 0 else nc.scalar
                eng.dma_start(
                    x_t[:, :c], xT[j * P:(j + 1) * P, m0:m0 + c].bitcast(F32R)
                )
                # round to f32r in place (read+write same AP is per-element safe)
                nc.vector.tensor_copy(x_t[:, :c], x_t[:, :c])
                xs.append(x_t)
            if ci == 0:
                # W rides both rings right behind the first chunk
                for j in range(KT):
                    eng = nc.sync if j % 2 == 0 else nc.scalar
                    eng.dma_start(w_f[:, j, :], w_v[j])
                    nc.vector.tensor_copy(w_r[:, j, :], w_f[:, j, :])

            ots = [outp.tile([P, CMAX], F32, tag=f"o{nt}", name=f"ot{nt}") for nt in range(NT)]
            for s0 in range(0, c, SUB):
                sw = min(SUB, c - s0)
                for nt in range(NT):
                    ps_o = pso.tile([P, SUB], F32, tag="pso")
                    for j in range(KT):
                        nc.tensor.matmul(
                            ps_o[:, :sw],
                            w_r[:, j, nt * P:(nt + 1) * P],
                            xs[j][:, s0:s0 + sw],
                            start=(j == 0),
                            stop=(j == KT - 1),
                        )
                    nc.vector.tensor_copy(ots[nt][:, s0:s0 + sw], ps_o[:, :sw])
            # flush the chunk: one DMA per n-tile on the SWDGE ring; the last
            # chunks ride the HWDGE rings (input traffic is done by then)
            for nt in range(NT):
                if ci >= len(CHUNKS) - 3:
                    eng = nc.sync if nt % 2 == 0 else nc.scalar
                else:
                    eng = nc.gpsimd
                eng.dma_start(outT[nt * P:(nt + 1) * P, m0:m0 + c], ots[nt][:, :c])
            m0 += c


def _build():
    nc = bacc.Bacc("TRN2", target_bir_lowering=False, debug=False, num_devices=G)
    xT = nc.dram_tensor("xT", [M, TOKENS], F32, kind="ExternalInput").ap()
    w = nc.dram_tensor("w", [M, N], F32, kind="ExternalInput").ap()
    outT = nc.dram_tensor("outT", [N, TOKENS], F32, kind="ExternalOutput").ap()
    with tile.TileContext(nc) as tc:
        _body(tc, nc, xT, w, outT)
    nc.compile()
    return nc


def _run(in_maps, **kwargs):
    if "nc" not in _CACHE:
        _CACHE["nc"] = _build()
    return run_bass_kernel_spmd(_CACHE["nc"], in_maps, list(range(G)), **kwargs)


def _in_maps(x, blocks):
    return [
        {
            "xT": np.ascontiguousarray(x[:, g * M:(g + 1) * M].T, dtype=np.float32),
            "w": np.ascontiguousarray(blocks[g], dtype=np.float32),
        }
        for g in range(G)
    ]


def kernel(x, blocks):
    x = np.asarray(x)
    blocks = np.asarray(blocks)
    res = _run(_in_maps(x, blocks))
    return np.concatenate(
        [res.results[g]["outT"].T for g in range(G)], axis=1
    ).astype(np.float32, copy=False)
